# revision 15
# baseline (speedup 1.0000x reference)
"""Multi-head self-attention (RoPE + softmax + out-proj) for Trainium2,
sharded over 8 NeuronCores: data-parallel over batch (4) x tensor-parallel
over heads (2 groups of 8). Each core computes q/k/v projections for its
head group, attention, and a partial output projection; the host sums the
two partials per batch and adds the bias.

v2 design (bf16 datapath, resident operands, balanced engines):
  - All matmul operands are bf16 (same PE stream rate as f32r, half the
    SBUF/DMA bytes; DVE elementwise gets the 2x/4x 16-bit perf modes).
  - x^T stays resident in SBUF (loaded once; no DRAM re-reads per pair);
    v is projected once into a per-head SBUF layout [key, head, 65] whose
    65th column is a ones column, so the attention matmul accumulates the
    softmax denominator for free (M=65 matmuls).
  - RoPE's rotate_half is four 32-partition shifted bf16 copies (4x DVE
    mode) off a single PSUM evacuation; the sign is folded into a
    host-negated sin table.
  - Scores are computed transposed (S^T[keys, queries]) with K=64 matmuls
    row-group-packed two heads at a time; exp runs on the scalar engine
    straight out of PSUM in 1024-wide instructions (scale folded in).
  - Softmax normalization: reciprocal_approx_fast on the denominator rows,
    one K=2 broadcast matmul per quarter to spread both heads' reciprocals
    across 128 partitions, and the PSUM evacuation of the attention output
    is fused with the normalize multiply (no staging spill).
  - Software pipeline: per 512-query quarter, 8 slots of
    {scores, exp, av(lagged one quarter), filler}; fillers carry the next
    pair's projections, the v projection (first quarter), and the output
    projection (last pair), keeping the PE dense so HAM stays warm.
"""

import numpy as np

import concourse.bass as bass
import concourse.mybir as mybir
import concourse.tile as tile

B, N, DIM, H, DH = 4, 2048, 1024, 16, 64
SCALE = DH**-0.5
N_CORES = 8
HG = 8  # heads per core
INNER = HG * DH  # 512
PAIRS = 4  # head pairs per core
NB = 4  # 512-wide query/key blocks
MB = 16  # 128-wide key blocks
KD = DIM // 128  # contraction chunks

F32 = mybir.dt.float32
F32R = mybir.dt.float32r
BF16 = mybir.dt.bfloat16
I32 = mybir.dt.int32
EXP = mybir.ActivationFunctionType.Exp

# Schraudolph exp offload to the vector engine: set of (p, qi, mb2, j)
# score tiles whose exp is computed as bitcast(int32(x*A + B)) on the DVE
# instead of the scalar engine (which is otherwise the bottleneck).
SCHRAUDOLPH = set()
SCH_A = SCALE * (1 << 23) / np.log(2.0)
SCH_B = float(127 * (1 << 23)) - 366393.0

MAX_WAITS = 1
WARMUP = True
SPLIT_FILLERS = False


def _split_excess_waits(nc):
    """This walrus build rejects >1 semaphore wait per instruction; hoist
    excess waits onto nops inserted before the instruction on its engine."""
    import bass_rust

    for f in nc.m.functions:
        for bb in f.blocks:
            il = bb.instructions
            i = 0
            while i < len(il):
                inst = il[i]
                si = inst.sync_info
                if si is not None and si.on_wait and len(si.on_wait) > MAX_WAITS:
                    waits = list(si.on_wait)
                    si.on_wait = waits[:MAX_WAITS]
                    rest = waits[MAX_WAITS:]
                    eng = nc.engines[inst.engine]
                    insert_at = i
                    for j in range(0, len(rest), MAX_WAITS):
                        b = eng.nop(nofuse=True, hint="wait_split")
                        ni = b.ins
                        tail = nc.cur_bb.bb.instructions
                        assert tail[-1] is ni
                        tail.pop()
                        nsi = ni.sync_info
                        if nsi is None:
                            ni.sync_info = bass_rust.SyncInfo(
                                on_wait=rest[j : j + MAX_WAITS], on_update=[]
                            )
                        else:
                            nsi.on_wait = rest[j : j + MAX_WAITS]
                        il.insert(insert_at, ni)
                        insert_at += 1
                        i += 1
                i += 1


class _FixedTileContext(tile.TileContext):
    def __exit__(self, exc_type, exc_val, exc_tb):
        res = super().__exit__(exc_type, exc_val, exc_tb)
        if exc_type is None:
            _split_excess_waits(self.nc)
        return res


def build_kernel():
    nc = bass.Bass()
    xT = nc.dram_tensor("xT", [DIM, N], BF16, kind="ExternalInput")
    wq = nc.dram_tensor("wq", [DIM, INNER], BF16, kind="ExternalInput")
    wk = nc.dram_tensor("wk", [DIM, INNER], BF16, kind="ExternalInput")
    wv = nc.dram_tensor("wv", [DIM, INNER], BF16, kind="ExternalInput")
    wo = nc.dram_tensor("wo", [INNER, DIM], BF16, kind="ExternalInput")
    cosT = nc.dram_tensor("cosT", [128, N], BF16, kind="ExternalInput")
    sinT = nc.dram_tensor("sinT", [128, N], BF16, kind="ExternalInput")
    onesd = nc.dram_tensor("onesd", [64, 128], F32R, kind="ExternalInput")
    out = nc.dram_tensor("out", [N, DIM], F32, kind="ExternalOutput")

    xTr = xT.rearrange("(c p) n -> p c n", p=128)
    wor = wo.rearrange("(c p) d -> p c d", p=128)

    with _FixedTileContext(nc) as tc:
        with (
            tc.tile_pool(name="const", bufs=1) as cpool,
            tc.tile_pool(name="w", bufs=2) as wpool,
            tc.tile_pool(name="qk", bufs=2) as qkpool,
            tc.tile_pool(name="rope", bufs=3) as rpool,
            tc.tile_pool(name="pt", bufs=20) as ptpool,
            tc.tile_pool(name="at", bufs=1) as at,
            tc.tile_pool(name="io", bufs=1) as iopool,
            tc.tile_pool(name="ps", space=bass.MemorySpace.PSUM, bufs=1) as ps,
        ):
            # ---- resident constants / activations ----
            # x first, split across the two HWDGE queues (sync + scalar) so
            # the projection chains aren't gated on one serialized queue
            onesblk = cpool.tile([64, 128], F32R, tag="onesblk")
            nc.sync.dma_start(onesblk[:], onesd[:])
            cos_t = cpool.tile([128, N], BF16, tag="cos")
            sin_t = cpool.tile([128, N], BF16, tag="sin")
            nc.sync.dma_start(cos_t[:], cosT[:])
            nc.scalar.dma_start(sin_t[:], sinT[:])
            x_sb = cpool.tile([128, KD, N], BF16, tag="x")
            x_engs = [nc.sync, nc.scalar, nc.gpsimd]
            for dc in range(KD):
                eng = x_engs[dc % 3]
                eng.dma_start(x_sb[:, dc, 0:1024], xTr[:, dc, 0:1024])
                eng.dma_start(x_sb[:, dc, 1024:N], xTr[:, dc, 1024:N])

            load_w0_done = []

            # v resident per (key-block, head, dh+ones): [128, 16, 8, 65].
            # Memset the whole tile to 1.0 up front: the projection evacs
            # overwrite columns 0-63 of each head, leaving column 64 as the
            # ones column that accumulates the softmax denominator.
            v_sb = cpool.tile([128, MB, HG, DH + 1], BF16, tag="vsb")
            nc.vector.memset(v_sb[:], 1.0)

            # persistent normalize staging: denominator rows 0 and 32 (other
            # rows stay 1.0 so the zero-weight broadcast rows see finite
            # values -- never NaN*0) and the Newton-iteration scratch
            den = at.tile([64, 512], F32, tag="den")
            nc.vector.memset(den[:], 1.0)
            rcp0 = at.tile([64, 512], I32, tag="rcp0")
            rcp_t = at.tile([64, 512], F32, tag="rcpt")
            rcp_u = at.tile([64, 512], F32, tag="rcpu")
            rcp1 = at.tile([64, 512], F32, tag="rcp1")
            rcp2 = at.tile([64, 512], F32R, tag="rcp2")

            # ---- per-pair q/k weight loads ----
            wtiles = {}

            def load_w(p):
                csl = slice(p * 128, (p + 1) * 128)
                ts = {}
                for nm, wd in (("q", wq), ("k", wk)):
                    t = wpool.tile([128, KD, 128], BF16, tag=f"w{nm}")
                    nc.gpsimd.dma_start(
                        t[:], wd.rearrange("(c p) i -> p c i", p=128)[:, :, csl]
                    )
                    ts[nm] = t
                wtiles[p] = ts

            # ---- projection block, split into two half-chain emitters so
            #      fillers stay fine-grained (~0.9us of PE work each) ----
            def proj_block_halves(p, nm, nb, tgt):
                st = {}

                def emit_a():
                    nsl = slice(nb * 512, (nb + 1) * 512)
                    pq = ps.tile([128, 512], F32, tag="pq", bufs=2)
                    st["pq"] = pq
                    wt = wtiles[p][nm]
                    for dc in range(4):
                        nc.tensor.matmul(
                            pq[:], wt[:, dc, :], x_sb[:, dc, nsl],
                            start=(dc == 0), stop=False,
                        )

                def emit_b():
                    nsl = slice(nb * 512, (nb + 1) * 512)
                    pq = st.pop("pq")
                    wt = wtiles[p][nm]
                    for dc in range(4, KD):
                        nc.tensor.matmul(
                            pq[:], wt[:, dc, :], x_sb[:, dc, nsl],
                            start=False, stop=(dc == KD - 1),
                        )
                    qsb = rpool.tile([128, 512], BF16, tag="qsb")
                    nc.vector.tensor_copy(qsb[:], pq[:])
                    # rotate_half: swap 32-row halves within each 64-row head
                    # block (sign folded into the host-negated sin table)
                    tmp = rpool.tile([128, 512], BF16, tag="tmp")
                    for g in range(4):
                        dst = slice(g * 32, (g + 1) * 32)
                        src = slice((g ^ 1) * 32, ((g ^ 1) + 1) * 32)
                        nc.vector.tensor_copy(tmp[dst, :], qsb[src, :])
                    nc.vector.tensor_mul(tmp[:], tmp[:], sin_t[:, nsl])
                    nc.vector.tensor_mul(tgt[:, nsl], qsb[:], cos_t[:, nsl])
                    nc.vector.tensor_add(tgt[:, nsl], tgt[:, nsl], tmp[:])

                return emit_a, emit_b

            # ---- v projection block halves: keys [i*128, (i+1)*128) ----
            def v_block_halves(i):
                st = {}

                def emit_a():
                    msl = slice(i * 128, (i + 1) * 128)
                    pv = ps.tile([128, 512], F32, tag="pq", bufs=2)
                    st["pv"] = pv
                    for dc in range(4):
                        nc.tensor.matmul(
                            pv[:], x_sb[:, dc, msl], wv_t[:, dc, :],
                            start=(dc == 0), stop=False,
                        )

                def emit_b():
                    msl = slice(i * 128, (i + 1) * 128)
                    pv = st.pop("pv")
                    for dc in range(4, KD):
                        nc.tensor.matmul(
                            pv[:], x_sb[:, dc, msl], wv_t[:, dc, :],
                            start=False, stop=(dc == KD - 1),
                        )
                    nc.vector.tensor_copy(v_sb[:, i, :, 0:DH], pv[:])

                return emit_a, emit_b

            # ---- output projection block (one 128-query row block, one
            #      512-wide dim half) ----
            otn = [
                at.tile([128, NB, 512], BF16, tag=f"otn{p}", name=f"otn{p}")
                for p in range(PAIRS)
            ]

            def outproj_block(nb, dh):
                def emit():
                    q4, r4 = divmod(nb, 4)
                    nsl = slice(nb * 128, (nb + 1) * 128)
                    po = ps.tile([128, 512], F32, tag="pq", bufs=2)
                    for c in range(PAIRS):
                        nc.tensor.matmul(
                            po[:],
                            otn[c][:, q4, r4 * 128 : (r4 + 1) * 128],
                            wo_t[:, c, dh * 512 : (dh + 1) * 512],
                            start=(c == 0), stop=(c == PAIRS - 1),
                        )
                    ost = iopool.tile([128, 512], F32, tag="ost", bufs=3)
                    nc.vector.tensor_copy(ost[:], po[:])
                    nc.sync.dma_start(out[nsl, dh * 512 : (dh + 1) * 512], ost[:])
                return emit

            # ---- attention pipeline ----
            fillers = []
            avq = []

            def drain_fillers(k):
                for _ in range(k):
                    if fillers:
                        fillers.pop(0)()

            def drain_avq(cap, lag=0, final=False):
                for _ in range(cap):
                    if not avq:
                        return
                    mb2_front = avq[0][0]
                    need = lag + (3 if (mb2_front == 0 and not final) else 0)
                    if len(avq) <= need:
                        return
                    avq.pop(0)[1]()

            def normalize(p, qi, qd):
                # stage both heads' denominator rows at partitions 0 and 32,
                # take the reciprocal there with a bit-magic seed plus two
                # Newton steps (stock DVE ops only; the per-op cost is
                # per-lane so the 33-row tile costs the same as one row),
                # then spread both rows across 128 partitions with one K=64
                # broadcast matmul.
                ot0, ot1 = qd["ot"]
                nc.vector.tensor_copy(den[0:1, :], ot0[64:65, :])
                nc.vector.tensor_copy(den[32:33, :], ot1[64:65, :])
                sl = slice(0, 33)
                nc.vector.tensor_scalar(
                    rcp0[sl, :], den[sl, :].bitcast(I32), 0x7EF312AC, -1,
                    mybir.AluOpType.subtract, mybir.AluOpType.mult,
                )
                r0 = rcp0[sl, :].bitcast(F32)
                nc.vector.tensor_mul(rcp_t[sl, :], den[sl, :], r0)
                nc.vector.tensor_scalar(
                    rcp_u[sl, :], rcp_t[sl, :], -1.0, 2.0,
                    mybir.AluOpType.mult, mybir.AluOpType.add,
                )
                nc.vector.tensor_mul(rcp1[sl, :], r0, rcp_u[sl, :])
                nc.vector.tensor_mul(rcp_t[sl, :], den[sl, :], rcp1[sl, :])
                nc.vector.tensor_scalar(
                    rcp_u[sl, :], rcp_t[sl, :], -1.0, 2.0,
                    mybir.AluOpType.mult, mybir.AluOpType.add,
                )
                nc.vector.tensor_mul(rcp2[sl, :], rcp1[sl, :], rcp_u[sl, :])
                bc = ps.tile([128, 512], F32, tag="pq", bufs=2)
                nc.tensor.matmul(
                    bc[:], onesblk[0:33, :], rcp2[0:33, :],
                    start=True, stop=True,
                )
                bcsb = at.tile([128, 512], F32, tag="bcsb", bufs=2)
                nc.vector.tensor_copy(bcsb[:], bc[:])
                # fused PSUM-evacuate + normalize (otn rows 0-63 = head 2p,
                # rows 64-127 = head 2p+1)
                nc.vector.tensor_mul(
                    otn[p][0:64, qi, :], ot0[0:64, :], bcsb[0:64, :]
                )
                nc.vector.tensor_mul(
                    otn[p][64:128, qi, :], ot1[0:64, :], bcsb[64:128, :]
                )

            def av_group(p, qi, mb2, qd):
                def emit():
                    if mb2 == 0:
                        qd["ot"] = [
                            ps.tile([128, 512], F32, tag="ot", bufs=2, name=f"ot{j}")
                            for j in range(2)
                        ]
                    pts = qd.pop(("pt", mb2))
                    for j in range(2):
                        pt, is_sch = pts[j]
                        for hm in range(2):
                            mb = 2 * mb2 + hm
                            rhs = pt[:, hm, :]
                            if is_sch:
                                rhs = rhs.bitcast(F32R)
                            nc.tensor.matmul(
                                qd["ot"][j][0:65, :],
                                v_sb[:, mb, 2 * p + j, :],
                                rhs,
                                start=(mb == 0), stop=(mb == MB - 1),
                            )
                    if mb2 == 7:
                        normalize(p, qi, qd)
                        if p == PAIRS - 1:
                            for r4 in range(4):
                                for dh in range(2):
                                    fillers.append(outproj_block(qi * 4 + r4, dh))
                        drain_fillers(1)
                return emit

            def halves(ab):
                if SPLIT_FILLERS:
                    return list(ab)
                a, b = ab

                def f():
                    a()
                    b()

                return [f]

            # prologue: pair-0 weights first on the gpsimd queue, then the
            # bulkier v/out weights
            load_w(0)
            wv_t = cpool.tile([128, KD, INNER], BF16, tag="wv")
            nc.gpsimd.dma_start(wv_t[:], wv.rearrange("(c p) i -> p c i", p=128))
            wo_t = cpool.tile([128, PAIRS, DIM], BF16, tag="wo")
            nc.gpsimd.dma_start(wo_t[:], wor[:])

            # warm-up matmuls on the (tiny, early-arriving) onesblk tile:
            # keep the PE busy while x streams in so HAM reaches full clock
            # before the real projection chains start
            warm = ps.tile([128, 2, 512], F32, tag="s", bufs=2, name="warm")
            for w in range(48 if WARMUP else 0):
                nc.tensor.matmul(
                    warm[:, 0, 0:128], onesblk[:], onesblk[:, 0:128].bitcast(F32R),
                    start=True, stop=True,
                )

            qT = {0: qkpool.tile([128, N], BF16, tag="qT", name="qT0")}
            kT = {0: qkpool.tile([128, N], BF16, tag="kT", name="kT0")}
            for nb in range(NB):
                a, b = proj_block_halves(0, "k", nb, kT[0])
                a(); b()
            a, b = proj_block_halves(0, "q", 0, qT[0])
            a(); b()
            # early fillers: v halves with the remaining qT0 halves woven in
            # so every quarter's inputs land ahead of its scores/av groups
            ev = [halves(v_block_halves(i)) for i in range(MB)]
            eq = [
                halves(proj_block_halves(0, "q", nb, qT[0]))
                for nb in range(1, NB)
            ]
            early = []
            for i in range(MB):
                early.extend(ev[i])
                if i in (1, 3, 5):
                    early.extend(eq[(i - 1) // 2])
            fillers.extend(early)

            for p in range(PAIRS):
                if p + 1 < PAIRS:
                    load_w(p + 1)
                    qT[p + 1] = qkpool.tile([128, N], BF16, tag="qT", name=f"qT{p+1}")
                    kT[p + 1] = qkpool.tile([128, N], BF16, tag="kT", name=f"kT{p+1}")
                    for nb in range(NB):
                        fillers.extend(
                            halves(proj_block_halves(p + 1, "k", nb, kT[p + 1]))
                        )
                    for nb in range(NB):
                        fillers.extend(
                            halves(proj_block_halves(p + 1, "q", nb, qT[p + 1]))
                        )
                for qi in range(4):
                    n0 = qi * 512
                    qd = {}
                    for mb2 in range(MB // 2):
                        pts = []
                        s_ts = [
                            ps.tile([128, 2, 512], F32, tag="s", bufs=2, name=f"s{j}")
                            for j in range(2)
                        ]
                        # alternate row groups (j0 at partitions 0-63, j1 at
                        # 64-127) so adjacent matmuls overlap in the PE array
                        for hm in range(2):
                            mb = 2 * mb2 + hm
                            msl = slice(mb * 128, (mb + 1) * 128)
                            for j in range(2):
                                psl = slice(64 * j, 64 * (j + 1))
                                nc.tensor.matmul(
                                    s_ts[j][:, hm, :],
                                    kT[p][psl, msl],
                                    qT[p][psl, n0 : n0 + 512],
                                    start=True, stop=True,
                                )
                        for j in range(2):
                            s_t = s_ts[j]
                            is_sch = (p, qi, mb2, j) in SCHRAUDOLPH
                            if is_sch:
                                pt = ptpool.tile(
                                    [128, 2, 512], I32, tag="pti", bufs=6
                                )
                                nc.vector.tensor_scalar(
                                    pt[:], s_t[:], SCH_A, SCH_B,
                                    mybir.AluOpType.mult, mybir.AluOpType.add,
                                )
                            else:
                                pt = ptpool.tile([128, 2, 512], BF16, tag="pt")
                                nc.scalar.activation(pt[:], s_t[:], EXP, scale=SCALE)
                            pts.append((pt, is_sch))
                        qd[("pt", mb2)] = pts
                        avq.append((mb2, av_group(p, qi, mb2, qd)))
                        # interleave: prologue drains fast; pairs 0-2 spread
                        # their 8 projection fillers across the pair; pair 3
                        # drains its outproj fillers every slot
                        if p == 0 and qi in (0, 1):
                            drain_fillers(3)
                        elif p == PAIRS - 1:
                            drain_fillers(1)
                        elif mb2 % 4 == 1:
                            drain_fillers(1)
                        if p == PAIRS - 1:
                            drain_avq(2, lag=1)
                        else:
                            drain_avq(1, lag=3)

            # tail: remaining attention groups, then output projection
            drain_avq(len(avq), final=True)
            drain_fillers(len(fillers))
            assert not avq and not fillers

    return nc


_CACHED = {}


def _get_kernel():
    if "nc" not in _CACHED:
        _CACHED["nc"] = build_kernel()
    return _CACHED["nc"]


def kernel(x, rotary_emb_x, Wq, Wkv, Wo, bo):
    import ml_dtypes
    from concourse.bass_utils import run_bass_kernel_spmd

    BF = ml_dtypes.bfloat16
    x = np.asarray(x, np.float32)
    rope = np.asarray(rotary_emb_x, np.float32)
    Wq = np.asarray(Wq, np.float32)
    Wkv = np.asarray(Wkv, np.float32)
    Wo = np.asarray(Wo, np.float32)
    bo = np.asarray(bo, np.float32)

    cosT = np.cos(rope).T  # [64, N]
    sinT = np.sin(rope).T
    cosT2 = np.concatenate([cosT, cosT], axis=0)
    sinT2 = np.concatenate([sinT, sinT], axis=0).copy()
    # fold rotate_half's sign into sin: the low half of each 64-row head
    # block multiplies -q_hi
    sinT2[0:32] = -sinT2[0:32]
    sinT2[64:96] = -sinT2[64:96]
    cosT2 = np.ascontiguousarray(cosT2.astype(BF))
    sinT2 = np.ascontiguousarray(sinT2.astype(BF))

    Wk_full = Wkv[:, : H * DH]
    Wv_full = Wkv[:, H * DH :]

    onesd = np.zeros((64, 128), np.float32)
    onesd[0, 0:64] = 1.0
    onesd[32, 64:128] = 1.0

    xTs = [np.ascontiguousarray(x[b].T.astype(BF)) for b in range(B)]
    in_maps = []
    for core in range(N_CORES):
        b, hg = divmod(core, 2)
        isl = slice(hg * INNER, (hg + 1) * INNER)
        in_maps.append(
            {
                "xT": xTs[b],
                "wq": np.ascontiguousarray(Wq[:, isl].astype(BF)),
                "wk": np.ascontiguousarray(Wk_full[:, isl].astype(BF)),
                "wv": np.ascontiguousarray(Wv_full[:, isl].astype(BF)),
                "wo": np.ascontiguousarray(Wo[isl, :].astype(BF)),
                "cosT": cosT2,
                "sinT": sinT2,
                "onesd": onesd,
            }
        )

    nc = _get_kernel()
    _CACHED["in_maps"] = in_maps
    res = run_bass_kernel_spmd(nc, in_maps, list(range(N_CORES)))
    outs = [res.results[i]["out"] for i in range(N_CORES)]
    full = np.stack(
        [outs[2 * b] + outs[2 * b + 1] + bo for b in range(B)], axis=0
    )
    return full
